# revision 49
# baseline (speedup 1.0000x reference)
"""Trainium2 Bass kernel for nn_DenseRED_SN (per-pixel spectral-norm dense reduce).

Math (full problem):
    w_mat = weight.reshape(H*W, C)
    sigma[p]  = ||w_mat[p, :]||_2                       (per-pixel L2 norm)
    out[b, 0, p] = (sum_c x[b, c, p] * w_mat[p, c]) / sigma[p] + bias[p]

Sharding: pixel-parallel over H across the 8 cores (32 image rows each).
Each core's slice of x / weight / bias is host-repacked (layout + f16
downcast of the streamed tensors — the kernel is DMA-roofline-bound, so
halving the streamed bytes halves runtime; f16 quantization adds ~3e-4
rel err vs the 2e-2 gate) into an SBUF-friendly "channel + pixel-half on
partitions" layout:
    partition p = h*64 + c   (h = pixel-half 0/1, c = channel)
    x_core[b, p, f]  = x[b, c, pix]  with pix = h*4096 + f
    w_core[p, f]     = w_mat[pix, c]

On-chip per core (all arithmetic on device):
    sq    = Square(w) per 2048-col chunk as each w chunk lands   (ScalarE)
    sig2  = ones_rep.T @ sq                                      (PE)
    sig   = Sqrt(sig2); rsig = 1/sig in place                    (ScalarE; DVE)
    per batch:      prod = x_b ⊙ w                               (VectorE)
                    acc += ones_blk_b.T @ prod chunks            (PE, accum)
    out   = acc ⊙ rsig (+ bias), stored f16                      (VectorE)

Timing structure (trace-driven; measured behaviors that shaped it):
  * exec window = [first Pool memset .. last teardown instruction]; a
    fixed ~8.5us epilogue (each engine retires ~57 event-semaphore
    entries) follows the last DMA regardless of program size.
  * x rides ONE HWDGE queue (sync ring) in per-batch 1MB contiguous
    transfers.  Pairing two batches per DMA lands transposed (the DMA
    pairs the flattened dest (partition-major) and src (batch-major)
    element streams in order).  The scalar ring starts ~1.6us later and
    is starved while the sync ring saturates (2MB routed there dribbled
    until ~65us), so only tiny consts + one drain store ride it.  All
    const loads are HWDGE (no gpsimd software-DGE semaphores).
  * all x dma_starts are emitted up front: interleaving them with the
    compute emission made the final transfers degrade to a ~0.9us-per-
    descriptor sem-paced dribble.  A milder form of that dribble still
    hits the ring's last entries in ~half of runs (process-level luck;
    run-to-run spread 62..72us, bimodal, also seen as a ~330 vs ~425
    GB/s stream mode in the f32 baseline era).
  * x is fully resident in one 16 MB SBUF tile (bufs=1): no buffer
    recycling, so the x stream issues with zero waits and never gaps.
  * single in-flight transfers only reach ~100-300 GB/s (descriptor
    streaming cap); saturation (~425 GB/s) needs several transfers in
    flight, so w+b0/b1 stream in 0.5MB halves at the head.
  * the sigma chain runs per-w-chunk as chunks land; the sigma matmuls
    are emitted after b2's (PE program order) by which time sq is long
    ready, so the PE (p-state ramp 0.65 -> 2.4 GHz) never stalls on them.
  * tail: b13/b14 in halves, b15 in quarters, so the DVE's backlog when
    the last pieces land is a quarter batch; acc is split into two PSUM
    band tiles so draining band A adds no false WAR against band-B
    matmuls (Tile tracks PSUM tiles coarsely), and each drain's f16
    store fires on its own ring as soon as its matmuls retire.

PSUM layout: the 32 output rows (16 batches x 2 pixel halves) for column
subgroup s (columns 1024s..1024s+1023) live on PSUM partitions 32s..32s+31,
written via matmul tile_position=(0, 32s); band j of the pair holds
columns 512j..512j+511 of each subgroup.
"""

import os

import numpy as np

H, W, C, B = 256, 256, 64, 16
NCORES = 8
ROWS = H // NCORES        # 32 image rows per core
PIX = ROWS * W            # 8192 pixels per core
HALF = PIX // 2           # 4096 (free-dim size; two pixel halves on partitions)
NCHUNK = 512              # matmul moving free dim (one PSUM bank of fp32)
SUB = 1024                # columns per partition-subgroup in the PSUM layout
NSUB = HALF // SUB        # 4 subgroups -> partition blocks 0/32/64/96
WCH = 2048                # w DMA/Square chunk (4 chunks)

_cache = {}


def _ensure_jax_platform():
    # bass2jax executes through the axon PJRT backend; make sure a
    # JAX_PLATFORMS=cpu pin from a caller does not hide the neuron devices.
    plat = os.environ.get("JAX_PLATFORMS")
    if plat is not None and "axon" not in plat and "neuron" not in plat:
        del os.environ["JAX_PLATFORMS"]


def _build_nc(use_f32r=True, with_bias=True):
    import concourse.bass as bass
    import concourse.tile as tile
    from concourse import bacc, mybir

    f32 = mybir.dt.float32
    f16 = mybir.dt.float16

    # Bacc (not raw Bass): its compile() pass lowers multi-wait instructions
    # into event-semaphore/NOP form — the raw 64B ISA slots hold only one
    # sync wait, so a plain Bass build fails walrus codegen on any
    # double-buffered pipeline.
    nc = bacc.Bacc("TRN2", target_bir_lowering=False, debug=False)

    # x and w stream from HBM as f16 (2e-2 rel-err gate leaves ~50x margin)
    x_d = nc.dram_tensor("x", [B, 128, HALF], f16, kind="ExternalInput")
    w_d = nc.dram_tensor("w", [128, HALF], f16, kind="ExternalInput")
    # products and the ones matrices must share a dtype for the PE; with x
    # already quantized to f16 there is nothing left for an f32 product
    # path to validate, so red_dt is f16 unconditionally (use_f32r kept
    # for the test harness's CLI compatibility)
    del use_f32r
    red_dt = f16
    oblk_d = nc.dram_tensor("ones_blk", [128, B * 32], f16, kind="ExternalInput")
    orep_d = nc.dram_tensor("ones_rep", [128, 32], f16, kind="ExternalInput")
    if with_bias:
        # host pre-packs bias (pure layout) into the [128, SUB] output layout
        bias_d = nc.dram_tensor("bias", [128, SUB], f32, kind="ExternalInput")
    out_d = nc.dram_tensor("out", [128, SUB], f16, kind="ExternalOutput")

    with tile.TileContext(nc) as tc:
        with (
            tc.tile_pool(name="const", bufs=1) as const_pool,
            tc.tile_pool(name="prod", bufs=4) as prod_pool,
            tc.tile_pool(name="accp", bufs=1, space="PSUM") as acc_pool,
            tc.tile_pool(name="sigp", bufs=1, space="PSUM") as sig_pool,
        ):
            # ---- w + small consts ride the scalar HWDGE ring (no gpsimd
            # software DGE: its DMASW semaphores lengthen the teardown) so
            # the sync ring starts streaming x immediately — w ahead of x
            # on the sync ring pushed the whole stream back by 1MB.  The
            # scalar ring is slow once the sync ring saturates (~100-150
            # GB/s), but 1.1MB still lands by ~12-16us, well before its
            # consumers; w goes in 0.25MB quarters so the first multiply
            # (which needs only quarter 0) unblocks ASAP.  Bulk data must
            # NOT ride it (2MB routed there dribbled until ~65us). ----
            ones_rep = const_pool.tile([128, 32], f16)
            nc.scalar.dma_start(out=ones_rep[:], in_=orep_d[:, :])
            w_sb = const_pool.tile([128, HALF], f16)
            for s in range(4):
                nc.scalar.dma_start(
                    out=w_sb[:, s * SUB:(s + 1) * SUB],
                    in_=w_d[:, s * SUB:(s + 1) * SUB],
                )
            ones_blk = const_pool.tile([128, B, 32], f16)
            nc.scalar.dma_start(out=ones_blk[:], in_=oblk_d[:, :])
            if with_bias:
                bias_sb = const_pool.tile([128, SUB], f32)
                nc.scalar.dma_start(out=bias_sb[:], in_=bias_d[:, :])

            # x: fully resident (16 MB); one write per region, no recycling
            x_all = const_pool.tile([128, B, HALF], f16)
            sq = const_pool.tile([128, HALF], red_dt)
            sig_ps = sig_pool.tile([128, SUB], f32)
            rsig = const_pool.tile([128, SUB], f32)
            out_sb = const_pool.tile([128, SUB], f16)
            # acc split into the two 512-column drain bands (one PSUM bank
            # each): chunk (s, j) accumulates into acc_j[32s:32s+32, :], so
            # draining band A adds no false WAR against band-B matmuls
            # (Tile tracks PSUM tiles coarsely)
            accA = acc_pool.tile([128, NCHUNK], f32, name="accA")
            accB = acc_pool.tile([128, NCHUNK], f32, name="accB")
            acc_band = (accA, accB)

            # ---- x stream (sync ring only: the scalar ring is starved
            # while the sync ring saturates — 2MB routed there dribbled
            # until ~65us and became the critical path, measured).  All
            # dma_starts are emitted up front: interleaving them with the
            # compute emission made the final transfers dribble. ----
            HH = HALF // 2
            QQ = HALF // 4

            def xdma(b, lo, hi):
                nc.sync.dma_start(out=x_all[:, b, lo:hi], in_=x_d[b, :, lo:hi])

            for b in (0, 1):
                xdma(b, 0, HH)
                xdma(b, HH, HALF)
            for b in range(2, B - 3):
                xdma(b, 0, HALF)
            for b in (B - 3, B - 2):
                xdma(b, 0, HH)
                xdma(b, HH, HALF)
            for q in range(4):
                xdma(B - 1, q * QQ, (q + 1) * QQ)

            # ---- sigma chain (ScalarE + PE, both otherwise idle).  The
            # Squares run per 2048-col chunk as w chunks land; the sigma
            # matmuls are emitted after b2's matmuls (PE program order), by
            # which time sq is long ready, so the PE never stalls on them.
            # Sqrt shares the act-table set with Square (sqrt_and_others):
            # one table load.  (Rsqrt is blocked in bass for accuracy.)
            # rsig temporarily holds sigma; inverted in place on the DVE in
            # a stream gap below. ----
            for s in range(HALF // WCH):
                nc.scalar.activation(
                    out=sq[:, s * WCH:(s + 1) * WCH],
                    in_=w_sb[:, s * WCH:(s + 1) * WCH],
                    func=mybir.ActivationFunctionType.Square,
                )

            def emit_sigma_mms():
                for s in range(NSUB):
                    for j in range(SUB // NCHUNK):
                        nc.tensor.matmul(
                            sig_ps[32 * s:32 * s + 32,
                                   j * NCHUNK:(j + 1) * NCHUNK],
                            ones_rep[:],
                            sq[:, s * SUB + j * NCHUNK:
                               s * SUB + (j + 1) * NCHUNK],
                            start=True,
                            stop=True,
                            tile_position=(0, 32 * s),
                        )
                nc.scalar.activation(
                    out=rsig[:], in_=sig_ps[:],
                    func=mybir.ActivationFunctionType.Sqrt,
                )

            def mm(b, prod, c, first):
                # prod holds batch b; chunk c feeds PSUM subgroup s=c//2,
                # drain band j=c%2
                s, j = divmod(c, SUB // NCHUNK)
                nc.tensor.matmul(
                    acc_band[j][32 * s:32 * s + 32, :],
                    ones_blk[:, b, :],
                    prod[:, c * NCHUNK:(c + 1) * NCHUNK],
                    start=first,
                    stop=False,
                    skip_group_check=True,
                    tile_position=(0, 32 * s),
                )

            def drain(j, deng):
                # acc band j is final for every partition: scale by
                # 1/sigma, add bias, store the band immediately.  Band j
                # holds original columns j*512..j*512+511 of each subgroup.
                lo, hi = j * NCHUNK, (j + 1) * NCHUNK
                nc.vector.tensor_mul(
                    out_sb[:, lo:hi], acc_band[j][:, :], rsig[:, lo:hi])
                if with_bias:
                    nc.vector.tensor_add(
                        out_sb[:, lo:hi], out_sb[:, lo:hi], bias_sb[:, lo:hi])
                deng.dma_start(out=out_d[:, lo:hi], in_=out_sb[:, lo:hi])

            def tt(prod, b, lo, hi):
                nc.vector.tensor_mul(
                    prod[:, lo:hi], x_all[:, b, lo:hi], w_sb[:, lo:hi])

            def new_prod(b):
                return prod_pool.tile([128, HALF], red_dt, tag="prod",
                                      name=f"prod_{b}")

            # ---- head: b0 per quarter (each quarter q needs only w
            # quarter q, so the first multiplies don't wait for all of w on
            # the slower scalar ring), b1 per half ----
            prod = new_prod(0)
            for q in range(4):
                tt(prod, 0, q * QQ, (q + 1) * QQ)
                mm(0, prod, 2 * q, first=True)
                mm(0, prod, 2 * q + 1, first=True)
            prod = new_prod(1)
            for v in range(2):
                tt(prod, 1, v * HH, (v + 1) * HH)
                for c in range(4 * v, 4 * v + 4):
                    mm(1, prod, c, first=False)

            # ---- steady: full batches ----
            for b in range(2, B - 3):
                prod = new_prod(b)
                tt(prod, b, 0, HALF)
                for c in range(8):
                    mm(b, prod, c, first=False)
                if b == 4:
                    # sigma matmuls slot in here: sq is ready by the time
                    # the PE drains b4's queue (w rides the slow scalar
                    # ring), and rsig is not needed until the drains
                    emit_sigma_mms()
                if b == 7:
                    # tail-only DVE work, emitted early so it fills a
                    # DMA-wait gap instead of delaying the drains
                    nc.vector.reciprocal_approx_fast(out=rsig[:], in_=rsig[:])

            # ---- tail: b13/b14 per half, b15 per quarter, so the DVE's
            # backlog when the last pieces land is a quarter batch; the
            # band-A drain+store fires right after the last band-A matmul,
            # band B immediately after ----
            for b in (B - 3, B - 2):
                prod = new_prod(b)
                for v in range(2):
                    tt(prod, b, v * HH, (v + 1) * HH)
                    for c in range(4 * v, 4 * v + 4):
                        mm(b, prod, c, first=False)
            prod = new_prod(B - 1)
            for q in range(4):
                tt(prod, B - 1, q * QQ, (q + 1) * QQ)
                mm(B - 1, prod, 2 * q, first=False)
                mm(B - 1, prod, 2 * q + 1, first=False)
            drain(0, nc.sync)
            drain(1, nc.scalar)

    nc.finalize()  # runs Bacc.compile(): reg alloc + multi-wait lowering
    return nc


def _ones_blk():
    if "ones_blk" not in _cache:
        o = np.zeros((128, B, 32), dtype=np.float32)
        p = np.arange(128)
        for b in range(B):
            o[p, b, 2 * b + (p // 64)] = 1.0
        _cache["ones_blk"] = np.ascontiguousarray(o.reshape(128, B * 32).astype(np.float16))
    return _cache["ones_blk"]


def _ones_rep():
    if "ones_rep" not in _cache:
        o = np.zeros((128, 32), dtype=np.float32)
        p = np.arange(128)[:, None]
        m = np.arange(32)[None, :]
        o[(m % 2) == (p // 64)] = 1.0
        _cache["ones_rep"] = np.ascontiguousarray(o.astype(np.float16))
    return _cache["ones_rep"]


def _shard_inputs(x, weight, bias, with_bias):
    """Host-side sharding/packing (layout + f16 downcast of the streamed
    tensors; the 2e-2 rel-err gate leaves ~50x margin over f16
    quantization).  Returns list of 8 input maps."""
    # downcast once up front so the per-core transpose-copies move half
    # the bytes
    x = np.asarray(x).astype(np.float16)
    weight = np.asarray(weight, dtype=np.float32)
    bias = np.asarray(bias, dtype=np.float32)
    w_mat = weight.reshape(H * W, C).astype(np.float16)
    bias_flat = bias.reshape(H * W)

    in_maps = []
    for k in range(NCORES):
        r0 = k * ROWS
        xs = x[:, :, r0:r0 + ROWS, :].reshape(B, C, PIX)
        # [B, C, 2, HALF] -> [B, 2, C, HALF] -> [B, 128, HALF]
        x_core = np.ascontiguousarray(
            xs.reshape(B, C, 2, HALF).transpose(0, 2, 1, 3)
        ).reshape(B, 128, HALF)

        ws = w_mat[r0 * W:(r0 + ROWS) * W, :]          # [PIX, C]
        # -> [2, HALF, C] -> [2, C, HALF] -> [128, HALF]
        w_core = np.ascontiguousarray(
            ws.reshape(2, HALF, C).transpose(0, 2, 1)
        ).reshape(128, HALF)

        m = {
            "x": x_core,
            "w": w_core,
            "ones_blk": _ones_blk(),
            "ones_rep": _ones_rep(),
        }
        if with_bias:
            # [2, NSUB, SUB] -> replicate over b -> row 32s + 2b + h
            v = bias_flat[r0 * W:(r0 + ROWS) * W].reshape(2, NSUB, SUB)
            bl = np.broadcast_to(v[None], (B, 2, NSUB, SUB))
            m["bias"] = np.ascontiguousarray(
                bl.transpose(2, 0, 1, 3).reshape(128, SUB))
        in_maps.append(m)
    return in_maps


def _unshard_output(results):
    out = np.zeros((B, 1, H, W), dtype=np.float32)
    for k in range(NCORES):
        # device layout: partition 32s + 2b + h holds columns s*SUB..(s+1)*SUB
        r = np.asarray(results[k]["out"]).astype(np.float32)   # [128, SUB]
        r = r.reshape(NSUB, B, 2, SUB).transpose(1, 2, 0, 3).reshape(B, PIX)
        out[:, 0, k * ROWS:(k + 1) * ROWS, :] = r.reshape(B, ROWS, W)
    return out


def _install_ntff_hook_shim():
    """This image lacks antenv.axon_hooks; bass_utils imports it whenever
    tracing is requested (including via a BASS_TRACE env var).  Recreate it
    with the ctypes-based hook from trn_boot so tracing degrades gracefully
    instead of crashing.  Idempotent and silent."""
    import sys
    try:
        import antenv.axon_hooks  # noqa: F401
        return
    except ImportError:
        pass
    try:
        import contextlib
        import ctypes
        import types

        mod = types.ModuleType("antenv.axon_hooks")
        state = {"hook": None}
        mod.set_axon_ntff_profile_hook = lambda h: state.__setitem__("hook", h)
        mod.get_axon_ntff_profile_hook = lambda: state["hook"]
        sys.modules["antenv.axon_hooks"] = mod

        so_path = "/opt/axon/libaxon_pjrt.so"
        lib = ctypes.CDLL(so_path)
        if not hasattr(lib, "axon_start_nrt_profile"):
            return
        lib.axon_start_nrt_profile.argtypes = [
            ctypes.POINTER(ctypes.c_int64), ctypes.c_size_t]
        lib.axon_start_nrt_profile.restype = ctypes.c_int64
        lib.axon_stop_nrt_profile.argtypes = [ctypes.c_char_p]
        lib.axon_stop_nrt_profile.restype = ctypes.c_int64

        @contextlib.contextmanager
        def _hook(output_dir, device_ids):
            import jax

            jax.devices()
            if device_ids:
                ids = (ctypes.c_int64 * len(device_ids))(*device_ids)
                rc = lib.axon_start_nrt_profile(ids, len(device_ids))
            else:
                rc = lib.axon_start_nrt_profile(None, 0)
            if rc != 0:
                raise RuntimeError(f"axon_start_nrt_profile rc={rc}")
            try:
                yield
            finally:
                lib.axon_stop_nrt_profile(str(output_dir).encode())

        mod.set_axon_ntff_profile_hook(_hook)
    except Exception:
        pass


def _run(inputs, trace=False, use_f32r=True):
    _ensure_jax_platform()
    _install_ntff_hook_shim()
    import concourse.bass_utils as _bu
    from concourse.bass_utils import run_bass_kernel_spmd

    # no cloud bucket in this container; keep trace artifacts local
    _bu.upload_artifacts = lambda tmpdir: tmpdir

    with_bias = bool(np.any(np.asarray(inputs["bias"])))
    key = ("nc", use_f32r, with_bias)
    if key not in _cache:
        _cache[key] = _build_nc(use_f32r=use_f32r, with_bias=with_bias)
    nc = _cache[key]

    in_maps = _shard_inputs(inputs["x"], inputs["weight"], inputs["bias"],
                            with_bias)
    res = run_bass_kernel_spmd(
        nc, in_maps, core_ids=list(range(NCORES)), trace=trace
    )
    return _unshard_output(res.results), res


def kernel(x, weight, bias):
    out, _ = _run({"x": x, "weight": weight, "bias": bias})
    return out
